# revision 59
# baseline (speedup 1.0000x reference)
"""AFNB (asymmetric fusion non-local block) Trainium2 kernel.

Data-parallel over batch: B=8 images, one per NeuronCore, no collectives.

Per-core algorithm (one image, H*W = N = 4096 pixels):
  pass 1 (low path):  kv = [relu(k_conv + kb); v_conv](low) computed
                      pixel-major in fp8 DoubleRow (2x PE rate); the K bias is
                      added by the DVE (psum + broadcast row) instead of a PE
                      matmul; pyramid-pooled (indicator matmul, fp8 DR) ->
                      kv_pool [110, 512] -> k_pool (PE transpose, stored fp8
                      at k/4) and v_poolT [110, 256] bf16.  V bias is folded
                      into the final output bias on the host (softmax rows sum
                      to 1, so it is exact).
  pass 2 (per 512-pixel tile): q = relu(q_conv(high)) in fp8 DR at q/4;
                      sim = (k/4).(q/4) (fp8 DR) = k.q/16 BN-folded; softmax
                      over the 110 pooled slots; ctx = v.softmax (bf16);
                      out = [W2_bf16 | W2_fp8]@high + A@ctx + bias with
                      A = bn_inv*(bn_w1@o_w)/32 and W2 = bn_inv*bn_w2 folded
                      on the host.

W2 mixed precision: output rows are sorted by bn_inv (host permutes rows of
W2/A/bias and un-permutes the result after the gather -- free).  Row chunk oc
(128 sorted rows) computes its K-suffix of PAIRS[oc]*256 input channels in fp8
DoubleRow (half the PE time of bf16) and the remaining prefix in bf16.  Low
bn_inv rows tolerate more fp8 (quantization error scales with the row norm);
the PAIRS allocation keeps the per-chunk max |error| <= ~1.45e-2 * out_scale
against the 2e-2 budget.  high is shipped twice: bf16 chunks 0..13 for the
bf16 matmuls and fp8(high*32) all 16 chunks for the q conv + fp8 W2 part
(also kills the on-chip bf16->fp8 cast that used to clog the DVE queue ahead
of psum drains).

Scale management (e4m3; subnormal below 2^-6): kw,vw,kb scaled x32; qw x256
and hi8 x32 (both undone by the q activation scale 1/32768); k_pool stored as
k/4 and q as q/4 so sim = k.q/16; psum scale for the out matmuls is
WSC = 8192 = (W2 bf16 x8192) = (W2 fp8 x256)@(hi8 x32) = (A x256)@(ctx x32).
Output stored fp16, upcast + row-unpermuted on the host.
"""

import numpy as np
import ml_dtypes

import concourse.bass as bass
import concourse.mybir as mybir
import concourse.tile as tile
from concourse import bacc
from concourse.bass_utils import run_bass_kernel_spmd
from concourse.masks import make_identity

BF = ml_dtypes.bfloat16
F8 = ml_dtypes.float8_e4m3fn
F16 = np.float16
F32 = np.float32
EPS = 1e-5
P = 128
N_CORES = 8
Cl, Ch, Cm, Co = 1024, 2048, 256, 2048
H = W = 64
NPIX = H * W            # 4096
M = 110                 # pooled slots: 1 + 9 + 36 + 64
M2 = 112                # M padded to a multiple of 16 (DoubleRow LDWEIGHTS stride rule)
KO = Cl // P            # 8  low-channel chunks
QO = Ch // P            # 16 high-channel chunks
OC = Co // P            # 16 out-channel chunks
PIX_T = 512             # pixel tile
NT = NPIX // PIX_T      # 8 tiles
PC = PIX_T // P         # 4 pixel chunks (128) per tile
KSC = 32.0              # K/V channel scale for fp8 range
QWS = 256.0             # q-weight fp8 scale
HISC = 32.0             # hi8 fp8 scale
AS8 = 256.0             # A fp8 scale
W8S = 256.0             # W2 fp8 scale (W8S * HISC = WSC)
WSC = 8192.0            # common psum scale for the out matmul group

# fp8 K-pairs (256 ch each) per sorted output chunk, low bn_inv first.  Tuned
# so each chunk's max W2-quantization error stays under ~1.45e-2 of out scale.
PAIRS = (8, 8, 8, 7, 6, 5, 3, 3, 3, 3, 2, 2, 2, 1, 1, 1)
NBF = tuple(16 - 2 * p for p in PAIRS)      # leading bf16 K chunks per oc
WOFF = tuple(int(x) for x in np.cumsum((0,) + NBF))   # into w2bf flat dim
# group processing order within a tile: natural order starts with the pure
# fp8 groups (oc 0-2), whose weights (W28 head) land first at startup, and
# reaches the bf16-heavy tail as the W2BF/hi_bf streams arrive
OCORD = tuple(range(16))
POFF = {}               # into w28 flat dim, laid out in OCORD order
_off = 0
for _oc in OCORD:
    POFF[_oc] = _off
    _off += PAIRS[_oc]
NW2BF = WOFF[-1]        # 130 bf16 weight chunks total
NW28 = _off             # 63 fp8 weight pair-chunks total
QOB = max(NBF)          # 14 bf16 hi chunks shipped

_cached = None
_last_results = None


def _pool_matrix(in_size, out_size):
    Pm = np.zeros((out_size, in_size), np.float64)
    for i in range(out_size):
        s = (i * in_size) // out_size
        e = -((-(i + 1) * in_size) // out_size)
        Pm[i, s:e] = 1.0 / (e - s)
    return Pm


def _build_pool_indicator():
    rows, areas = [], []
    for s in (1, 3, 6, 8):
        Ph = _pool_matrix(H, s) != 0
        Pw = _pool_matrix(W, s) != 0
        for i in range(s):
            for j in range(s):
                ind = np.outer(Ph[i], Pw[j]).reshape(-1)
                rows.append(ind.astype(np.float32))
                areas.append(ind.sum())
    return np.stack(rows), 1.0 / np.asarray(areas, np.float64)


def _chunk_T(w, chunks):
    """[rows, cols] -> SBUF layout [128, chunks, rows] with [p,o,m]=w[m,o*128+p]."""
    rows, cols = w.shape
    assert cols == chunks * P
    return np.ascontiguousarray(w.T.reshape(chunks, P, rows).transpose(1, 0, 2))


def _prep_weights(inp):
    f64 = lambda k: np.asarray(inp[k], np.float64)
    inv_q = f64("q_g") / np.sqrt(f64("q_v") + EPS)
    qw = (inv_q[:, None] * f64("q_w")) / 16.0        # fold BN + 1/sqrt(256)
    qb = (f64("q_b") - f64("q_m") * inv_q) / 16.0
    inv_k = f64("k_g") / np.sqrt(f64("k_v") + EPS)
    kw = inv_k[:, None] * f64("k_w")
    kb = f64("k_b") - f64("k_m") * inv_k
    bn_w1 = f64("bn_w")[:, :Ch]
    bn_w2 = f64("bn_w")[:, Ch:]
    inv_bn = f64("bn_g") / np.sqrt(f64("bn_v") + EPS)
    perm = np.argsort(inv_bn, kind="stable")         # rows sorted by bn_inv
    A = (inv_bn[:, None] * (bn_w1 @ f64("o_w")))[perm]       # [2048, 256]
    W2 = (inv_bn[:, None] * bn_w2)[perm]                     # [2048, 2048]
    # v_b folded here: ctx = softmax@(v+v_b) = softmax@v + v_b exactly
    bias_out = (inv_bn * (bn_w1 @ (f64("o_w") @ f64("v_b") + f64("o_b")))
                + f64("bn_b") - f64("bn_m") * inv_bn)[perm]

    kv_w = np.concatenate([kw, f64("v_w")], 0) * KSC  # [512, 1024] x32
    ind, area_recip = _build_pool_indicator()         # [110, 4096], [110]
    ind = np.concatenate([ind, np.zeros((M2 - M, NPIX), np.float32)], 0)

    # bf16 W2 prefix chunks, ragged per oc: [p, flat, c] = W2[oc*128+c, o*128+p]
    w2bf = np.zeros((P, NW2BF, P), np.float64)
    # fp8 W2 suffix pairs: [p, flatpair, s, m] = W2[oc*128+m, (2jj+s)*128+p]
    w28 = np.zeros((P, NW28, 2, P), np.float64)
    for oc in range(OC):
        rows = W2[oc * P:(oc + 1) * P]               # [128, 2048]
        for o in range(NBF[oc]):
            w2bf[:, WOFF[oc] + o, :] = rows[:, o * P:(o + 1) * P].T
        for j in range(PAIRS[oc]):
            jj = NBF[oc] // 2 + j
            for s in range(2):
                w28[:, POFF[oc] + j, s, :] = rows[:, (2 * jj + s) * P:(2 * jj + s + 1) * P].T

    return {
        "kvw8": _chunk_T(kv_w, KO).astype(F8),                        # [128, 8, 512]
        "kb": np.ascontiguousarray(np.stack([(kb * KSC), np.zeros_like(kb)])[None]).astype(F8),  # [1, 2, 256]
        "qw8": _chunk_T(qw * QWS, QO).astype(F8),                     # [128, 16, 256]
        "qb": np.ascontiguousarray((qb / 4.0).reshape(2, P).T).astype(F32),  # [128, 2]
        "AT": _chunk_T(A * AS8, 2).astype(F8),                        # [128, 2, 2048]
        "W2BF": np.ascontiguousarray(w2bf * WSC).astype(BF),          # [128, 130, 128]
        "W28": np.ascontiguousarray(w28 * W8S).astype(F8),            # [128, 63, 2, 128]
        "bout": np.ascontiguousarray(bias_out.reshape(OC, P).T).astype(F32),  # [128, 16]
        "ind8": _chunk_T(ind, NPIX // P).astype(F8),                  # [128, 32, 112]
        "area": np.ascontiguousarray(area_recip[:, None]).astype(F32),  # [110, 1]
    }, perm


def build_bass():
    bf = mybir.dt.bfloat16
    f8 = mybir.dt.float8e4
    f16 = mybir.dt.float16
    f32 = mybir.dt.float32
    DR = mybir.MatmulPerfMode.DoubleRow
    ACT = mybir.ActivationFunctionType
    ALU = mybir.AluOpType
    nc = bacc.Bacc()
    low_e = nc.declare_dram_parameter("low", [P, NT, KO, PIX_T], f8, isOutput=False)
    high_e = nc.declare_dram_parameter("high", [P, NT, QOB, PIX_T], bf, isOutput=False)
    hi8_e = nc.declare_dram_parameter("high8", [P, NT, 2, PIX_T], f8, isOutput=False)
    kvw_e = nc.declare_dram_parameter("kvw8", [P, KO, 512], f8, isOutput=False)
    kb_e = nc.declare_dram_parameter("kb", [1, 2, Cm], f8, isOutput=False)
    qw_e = nc.declare_dram_parameter("qw8", [P, QO, Cm], f8, isOutput=False)
    qb_e = nc.declare_dram_parameter("qb", [P, 2], f32, isOutput=False)
    at_e = nc.declare_dram_parameter("AT", [P, 2, Co], f8, isOutput=False)
    w2bf_e = nc.declare_dram_parameter("W2BF", [P, NW2BF, P], bf, isOutput=False)
    w28_e = nc.declare_dram_parameter("W28", [P, NW28, 2, P], f8, isOutput=False)
    bo_e = nc.declare_dram_parameter("bout", [P, OC], f32, isOutput=False)
    ind_e = nc.declare_dram_parameter("ind8", [P, NPIX // P, M2], f8, isOutput=False)
    ar_e = nc.declare_dram_parameter("area", [M, 1], f32, isOutput=False)
    out_e = nc.declare_dram_parameter("out", [Co, NPIX], f16, isOutput=True)

    out_r = out_e[:].rearrange("(o p) n -> o p n", p=P)    # [16, 128, 4096]

    with tile.TileContext(nc) as tc:
        with (
            tc.tile_pool(name="consts", bufs=1) as consts,
            tc.tile_pool(name="lobf", bufs=6) as lobf_p,
            tc.tile_pool(name="kvt", bufs=2) as kvt_p,
            tc.tile_pool(name="hibf", bufs=3) as hibf_p,
            tc.tile_pool(name="hi8b", bufs=3) as hi8_p,
            tc.tile_pool(name="qsb", bufs=2) as q_p,
            tc.tile_pool(name="esb", bufs=1) as e_p,
            tc.tile_pool(name="ensb", bufs=1) as en_p,
            tc.tile_pool(name="rsb", bufs=1) as r_p,
            tc.tile_pool(name="ctxsb", bufs=2) as ctx_p,
            tc.tile_pool(name="osb", bufs=4) as o_p,
            tc.tile_pool(name="psbig", bufs=2, space="PSUM") as psbig_p,
            tc.tile_pool(name="pso", bufs=5, space="PSUM") as pso_p,
            tc.tile_pool(name="psmall", bufs=1, space="PSUM") as psmall_p,
        ):
            # PE warmup: junk matmuls so the HAM clock gate opens (1.2->2.4GHz
            # after ~3.4us of activity) during the DMA-wait window at start
            junk = consts.tile([P, PIX_T], bf)
            nc.vector.memset(junk, 0.015)
            pswarm = psbig_p.tile([P, PIX_T], f32, tag="big")
            for _ in range(10):
                nc.tensor.matmul(pswarm, junk[:, 0:P], junk, start=True,
                                 stop=True, skip_group_check=True)

            # pass-1 streams issued up front, STRIPED across the three DMA
            # trigger queues (sync, scalar, gpsimd; each ~110 B/ns) in
            # need-order: kvw/lo0 pairs first, ind pair-chunks paced to the
            # in-order pool matmuls, later lo tiles behind.  Pre-hoisted
            # triggers are safe even on scalar: they execute before any
            # compute lands in its strict FIFO.
            kb_sb = consts.tile([1, 2, Cm], f8)
            nc.sync.dma_start(kb_sb, kb_e[:])
            kvw_sb = consts.tile([P, KO, 512], f8)
            lo_tiles = [lobf_p.tile([P, KO, PIX_T], f8, name="lo8")
                        for i in range(6)]
            ind_sb = consts.tile([P, NPIX // P, M2], f8)
            lo0 = lo_tiles[0]
            lo0_e = low_e[:][:, 0]
            kvw_d = kvw_e[:]
            nc.sync.dma_start(kvw_sb[:, 0:2, :], kvw_d[:, 0:2, :])
            nc.scalar.dma_start(kvw_sb[:, 2:4, :], kvw_d[:, 2:4, :])
            nc.gpsimd.dma_start(kvw_sb[:, 4:6, :], kvw_d[:, 4:6, :])
            nc.sync.dma_start(lo0[:, 0:2, :], lo0_e[:, 0:2, :])
            nc.scalar.dma_start(lo0[:, 2:4, :], lo0_e[:, 2:4, :])
            nc.gpsimd.dma_start(lo0[:, 4:6, :], lo0_e[:, 4:6, :])
            nc.sync.dma_start(kvw_sb[:, 6:8, :], kvw_d[:, 6:8, :])
            nc.scalar.dma_start(lo0[:, 6:8, :], lo0_e[:, 6:8, :])
            nc.gpsimd.dma_start(ind_sb[:, 0:4, :], ind_e[:][:, 0:4, :])
            nc.sync.dma_start(lo_tiles[1], low_e[:][:, 1])
            nc.gpsimd.dma_start(lo_tiles[2], low_e[:][:, 2])
            nc.gpsimd.dma_start(ind_sb[:, 4:12, :], ind_e[:][:, 4:12, :])
            nc.scalar.dma_start(lo_tiles[3], low_e[:][:, 3])
            nc.gpsimd.dma_start(ind_sb[:, 12:20, :], ind_e[:][:, 12:20, :])
            nc.sync.dma_start(lo_tiles[4], low_e[:][:, 4])
            nc.sync.dma_start(lo_tiles[5], low_e[:][:, 5])
            nc.gpsimd.dma_start(ind_sb[:, 20:32, :], ind_e[:][:, 20:32, :])
            ar_sb = consts.tile([M, 1], f32)
            nc.gpsimd.dma_start(ar_sb, ar_e[:])
            # tile-0 bf16 hi staged pre-loop: the scalar queue is clear of
            # compute here, and pass-1 emits no further scalar triggers
            hi_tiles, hi8_tiles = {}, {}

            def stage_hibf(t):
                hi_bf = hibf_p.tile([P, QOB, PIX_T], bf, name="hi_bf")
                nc.scalar.dma_start(hi_bf, high_e[:][:, t])
                hi_tiles[t] = hi_bf

            # tile-0 bf16 hi rides the scalar queue behind the lo stream;
            # needed only from tile-0's 4th group (oc3), well after it lands
            stage_hibf(0)

            ones8 = consts.tile([1, 2, P], f8)    # K-bias DR seed lhsT
            nc.vector.memset(ones8[:, 0, :], 1.0)
            nc.vector.memset(ones8[:, 1, :], 0.0)
            ones1m = consts.tile([1, M], bf)      # psr broadcast lhsT
            nc.vector.memset(ones1m, 1.0)
            ones_m = consts.tile([M, 1], bf)      # denominator lhsT
            nc.vector.memset(ones_m, 1.0)
            ident = consts.tile([P, P], f32)
            make_identity(nc, ident)

            kvpool_f32 = consts.tile([M, 512], f32)
            v_poolT = consts.tile([M, Cm], bf)
            k_pool8 = consts.tile([P, 2, M2], f8)
            nc.vector.memset(k_pool8, 0.0)

            # ---------------- pass 1: low -> pooled K/V (fp8 DR) ----------------
            pool_acc = pso_p.tile([M2, 512], f32, name="pool_acc", tag="o")
            kvt8 = None
            kvt_gate = None
            for dt_ in range(NT):
                lo8 = lo_tiles[dt_]
                # tiles 6/7 reuse ring slots 0/1 once those readers exist; the
                # triggers land on queues that are idle by then
                if dt_ == 1:
                    lo_tiles.append(lobf_p.tile([P, KO, PIX_T], f8, name="lo8"))
                    nc.sync.dma_start(lo_tiles[6], low_e[:][:, 6])
                elif dt_ == 2:
                    lo_tiles.append(lobf_p.tile([P, KO, PIX_T], f8, name="lo8"))
                    nc.gpsimd.dma_start(lo_tiles[7], low_e[:][:, 7])
                for tt in range(PC):
                    t = dt_ * PC + tt
                    ps = psbig_p.tile([P, 512], f32, tag="big")
                    for o2 in range(KO // 2):
                        nc.tensor.matmul(
                            ps, lo8[:, 2 * o2:2 * o2 + 2, tt * P:(tt + 1) * P],
                            kvw_sb[:, 2 * o2:2 * o2 + 2, :],
                            start=(o2 == 0), stop=(o2 == KO // 2 - 1),
                            perf_mode=DR, skip_group_check=True)
                        if o2 == 0:   # K-bias into the zeroed psum (cols 0:256)
                            nc.tensor.matmul(ps[:, 0:Cm], ones8[:, 0:2, :],
                                             kb_sb[:, 0:2, :],
                                             start=False, stop=False, perf_mode=DR,
                                             skip_group_check=True)
                    half = t % 2
                    if half == 0:
                        kvt8 = kvt_p.tile([P, 2, 512], f8)
                    nc.scalar.activation(kvt8[:, half, 0:Cm], ps[:, 0:Cm], ACT.Relu)
                    nc.vector.tensor_copy(kvt8[:, half, Cm:512], ps[:, Cm:512])
                    if half == 1:
                        nc.tensor.matmul(pool_acc, ind_sb[:, t - 1:t + 1, :], kvt8,
                                         start=(t == 1), stop=(t == NPIX // P - 1),
                                         perf_mode=DR, skip_group_check=True)
                        if t == 11:
                            kvt_gate = kvt8

            # hi8: only the fp8-exclusive chunks 14/15 come over DMA; chunks
            # 0..13 are cast on-chip from hi_bf (x32, bf16->fp8) in pieces
            # interleaved between groups so they never clog the DVE FIFO
            CAST_SL = ((0, 4), (4, 8), (8, 11), (11, 14))

            def stage_hi8(t):
                hi8t = hi8_p.tile([P, QO, PIX_T], f8, name="hi8t")
                nc.sync.dma_start(hi8t[:, 14:16, :], hi8_e[:][:, t])
                hi8_tiles[t] = hi8t

            def cast_hi8(t, piece):
                a, b = CAST_SL[piece]
                nc.vector.tensor_scalar_mul(hi8_tiles[t][:, a:b, :],
                                            hi_tiles[t][:, a:b, :], HISC)

            stage_hi8(0)

            # pass-2 weights in need-order: tile 0 walks groups in OCORD, so
            # W28 heads (OCORD[:7], pairs 0:45) go before the tail; the bf16
            # W2 stream is trickled through tile 0's body below
            hibf1 = hibf_p.tile([P, QOB, PIX_T], bf, name="hi_bf")
            nc.sync.dma_start(hibf1[:, 0:7, :], high_e[:][:, 1, 0:7, :])
            nc.scalar.dma_start(hibf1[:, 7:QOB, :], high_e[:][:, 1, 7:QOB, :])
            hi_tiles[1] = hibf1
            at_sb = consts.tile([P, 2, Co], f8)
            nc.sync.dma_start(at_sb, at_e[:])
            w2bf_sb = consts.tile([P, NW2BF, P], bf)
            nc.sync.dma_start(w2bf_sb[:, 0:6], w2bf_e[:][:, 0:6])
            nc.sync.dma_start(w2bf_sb[:, 6:22], w2bf_e[:][:, 6:22])
            qw_sb = consts.tile([P, QO, Cm], f8)
            nc.gpsimd.dma_start(qw_sb, qw_e[:])
            qb_sb = consts.tile([P, 2], f32)
            nc.gpsimd.dma_start(qb_sb, qb_e[:])
            bo_sb = consts.tile([P, OC], f32)
            nc.gpsimd.dma_start(bo_sb, bo_e[:])
            w28_sb = consts.tile([P, NW28, 2, P], f8)
            nc.gpsimd.dma_start(w28_sb[:, 0:24], w28_e[:][:, 0:24])
            nc.gpsimd.dma_start(w28_sb[:, 24:45], w28_e[:][:, 24:45])
            nc.gpsimd.dma_start(w28_sb[:, 45:NW28], w28_e[:][:, 45:NW28])
            nc.gpsimd.dma_start(w2bf_sb[:, 42:64], w2bf_e[:][:, 42:64])

            def emit_q(t):
                """fp8 DR q conv for tile t -> q/4 in fp8 [P, 2, PIX_T]."""
                hi8t = hi8_tiles[t]
                q8 = q_p.tile([P, 2, PIX_T], f8)
                for j in range(2):
                    psq = psbig_p.tile([P, PIX_T], f32, tag="big")
                    for o2 in range(QO // 2):
                        nc.tensor.matmul(
                            psq, qw_sb[:, 2 * o2:2 * o2 + 2, j * P:(j + 1) * P],
                            hi8t[:, 2 * o2:2 * o2 + 2, :],
                            start=(o2 == 0), stop=(o2 == QO // 2 - 1),
                            perf_mode=DR)
                    nc.scalar.activation(q8[:, j, :], psq, ACT.Relu,
                                         bias=qb_sb[:, j:j + 1],
                                         scale=1.0 / (4.0 * QWS * HISC))
                return q8

            for pc_ in range(4):
                cast_hi8(0, pc_)
            q_next = emit_q(0)

            # epilogue: scale by 1/area, split V (bf16) / K (fp8 at k/4 via
            # transpose + scaled copy); PE transposes overlap tile-0 q conv
            nc.vector.tensor_scalar_mul(kvpool_f32, pool_acc[0:M, :], ar_sb)
            nc.scalar.activation(v_poolT, kvpool_f32[:, Cm:512], ACT.Copy)
            for j in range(2):
                pst = psbig_p.tile([P, M], f32, tag="big")
                nc.tensor.transpose(pst, kvpool_f32[:, j * P:(j + 1) * P],
                                    ident[:M, :M])
                nc.scalar.activation(k_pool8[:, j, 0:M], pst, ACT.Identity,
                                     scale=1.0 / (4.0 * KSC))

            # per-oc W2 op lists: bf16 prefix chunks + fp8 DR suffix pairs
            OPS = []
            for oc in range(OC):
                OPS.append([("bf", o) for o in range(NBF[oc])]
                           + [("dr", NBF[oc] // 2 + j) for j in range(PAIRS[oc])])

            # ---------------- pass 2: per pixel tile ----------------
            # ctx for tile t+1 is produced inside tile t (chain links spread
            # between groups, hiding their serial latency behind queued PE
            # work), so each group runs contiguously: its W2 matmuls, the
            # A@ctx close, the split drain and the output DMA.  Psum banks
            # recycle a full 4-group period after their drain.

            def chain_head(q8):
                psim = psmall_p.tile([M2, PIX_T], f32, tag="s", name="psim")
                nc.tensor.matmul(psim, k_pool8[:, 0:2, :], q8[:, 0:2, :],
                                 start=True, stop=True, perf_mode=DR,
                                 skip_group_check=True)
                e_sb = e_p.tile([M, PIX_T], bf, name="e_sb")
                nc.scalar.activation(e_sb, psim[0:M, :], ACT.Exp)
                return e_sb

            def chain_mid1(e_sb):
                psd = psmall_p.tile([1, PIX_T], f32, tag="s", name="psd")
                nc.tensor.matmul(psd, ones_m, e_sb, start=True, stop=True,
                                 skip_group_check=True)
                r_sb = r_p.tile([1, PIX_T], f32, name="r_sb")
                nc.vector.reciprocal_approx_fast(out=r_sb, in_=psd)
                r_bf = r_p.tile([1, PIX_T], bf, name="r_bf")
                nc.scalar.activation(r_bf, r_sb, ACT.Copy)
                return r_bf

            def chain_mid2(e_sb, r_bf):
                psr = psmall_p.tile([M, PIX_T], f32, tag="s", name="psr")
                nc.tensor.matmul(psr, ones1m, r_bf, start=True, stop=True,
                                 skip_group_check=True)
                en_sb = en_p.tile([M, PIX_T], bf, name="en_sb")
                nc.vector.tensor_mul(en_sb, e_sb, psr)
                return en_sb

            def chain_tail1(en_sb):
                psc0 = psbig_p.tile([P, PIX_T], f32, tag="big", name="psc0")
                nc.tensor.matmul(psc0, v_poolT[:, 0:P], en_sb,
                                 start=True, stop=True, skip_group_check=True)
                psc1 = psbig_p.tile([P, PIX_T], f32, tag="big", name="psc1")
                nc.tensor.matmul(psc1, v_poolT[:, P:2 * P], en_sb,
                                 start=True, stop=True, skip_group_check=True)
                return psc0, psc1

            def chain_tail2(psc0, psc1):
                ctx_sb = ctx_p.tile([P, 2, PIX_T], f8, name="ctx_sb")
                nc.vector.tensor_copy(ctx_sb[:, 0, :], psc0)
                nc.vector.tensor_copy(ctx_sb[:, 1, :], psc1)
                return ctx_sb

            # tile-0 chain in the prologue (overlaps the pass-1 tail / q conv)
            e_nx = chain_head(q_next)
            r_nx = chain_mid1(e_nx)
            en_nx = chain_mid2(e_nx, r_nx)
            pc0, pc1 = chain_tail1(en_nx)
            ctx_next = chain_tail2(pc0, pc1)

            for t in range(NT):
                sl = slice(t * PIX_T, (t + 1) * PIX_T)
                hi_bf = hi_tiles[t]
                hi8t = hi8_tiles[t]
                if t + 2 < NT:
                    stage_hibf(t + 2)
                q8 = q_next
                ctx_sb = ctx_next
                last = t + 1 >= NT

                def do_group(oc, gi, hi_bf=hi_bf, hi8t=hi8t, ctx_sb=ctx_sb,
                             sl=sl):
                    pso = pso_p.tile([P, PIX_T], f32, name="pso", tag="o")
                    first = True
                    for kind, idx in OPS[oc]:
                        if kind == "bf":
                            nc.tensor.matmul(pso, w2bf_sb[:, WOFF[oc] + idx, :],
                                             hi_bf[:, idx, :],
                                             start=first, stop=False,
                                             skip_group_check=True)
                        else:
                            j = idx - NBF[oc] // 2
                            nc.tensor.matmul(pso, w28_sb[:, POFF[oc] + j],
                                             hi8t[:, 2 * idx:2 * idx + 2, :],
                                             start=first, stop=False,
                                             perf_mode=DR, skip_group_check=True)
                        first = False
                    nc.tensor.matmul(pso, at_sb[:, 0:2, oc * P:(oc + 1) * P],
                                     ctx_sb[:, 0:2, :],
                                     start=False, stop=True, perf_mode=DR,
                                     skip_group_check=True)
                    o_sb = o_p.tile([P, PIX_T], f16)
                    # full-width drains on alternating engines: keeps each
                    # FIFO sparse so the softmax-chain ops are not delayed
                    if gi % 2 == 1:
                        nc.scalar.activation(o_sb, pso, ACT.Identity,
                                             bias=bo_sb[:, oc:oc + 1],
                                             scale=1.0 / WSC)
                        nc.gpsimd.dma_start(out_r[oc][:, sl], o_sb)
                    else:
                        nc.vector.tensor_scalar(o_sb, pso, 1.0 / WSC,
                                                bo_sb[:, oc:oc + 1],
                                                ALU.mult, ALU.add)
                        nc.sync.dma_start(out_r[oc][:, sl], o_sb)

                for gi, oc in enumerate(OCORD):
                    do_group(oc, gi)
                    if t == 0:   # trickle the bf16 W2 stream behind tile 0
                        if gi == 0:
                            nc.sync.dma_start(w2bf_sb[:, 22:42],
                                              w2bf_e[:][:, 22:42])
                        elif gi == 1:
                            nc.gpsimd.dma_start(w2bf_sb[:, 64:88],
                                                w2bf_e[:][:, 64:88])
                        elif gi == 2:
                            nc.sync.dma_start(w2bf_sb[:, 88:108],
                                              w2bf_e[:][:, 88:108])
                        elif gi == 5:
                            nc.gpsimd.dma_start(w2bf_sb[:, 108:130],
                                                w2bf_e[:][:, 108:130])
                    if not last:
                        if gi == 0:
                            stage_hi8(t + 1)
                        elif 1 <= gi <= 4:
                            cast_hi8(t + 1, gi - 1)
                        elif gi == 5:
                            q_next = emit_q(t + 1)
                        elif gi == 6:
                            e_nx = chain_head(q_next)
                        elif gi == 8:
                            r_nx = chain_mid1(e_nx)
                        elif gi == 10:
                            en_nx = chain_mid2(e_nx, r_nx)
                        elif gi == 12:
                            pc0, pc1 = chain_tail1(en_nx)
                        elif gi == 14:
                            ctx_next = chain_tail2(pc0, pc1)
    nc.finalize()
    return nc


def kernel(**inputs):
    global _cached, _last_results
    if _cached is None:
        _cached = build_bass()
    nc = _cached
    wts, perm = _prep_weights(inputs)
    # pack [C, H*W] -> [p, tile, o, pix] so each per-tile DMA is contiguous
    low = np.ascontiguousarray(
        np.asarray(inputs["low_feats"], F32).reshape(N_CORES, KO, P, NT, PIX_T)
        .transpose(0, 2, 3, 1, 4).astype(F8))
    high_f = (np.asarray(inputs["high_feats"], F32)
              .reshape(N_CORES, QO, P, NT, PIX_T).transpose(0, 2, 3, 1, 4))
    high = np.ascontiguousarray(high_f[:, :, :, 0:QOB, :].astype(BF))
    high8 = np.ascontiguousarray((high_f[:, :, :, 14:16, :] * HISC).astype(F8))
    in_maps = [dict(wts, low=low[i], high=high[i], high8=high8[i])
               for i in range(N_CORES)]
    res = run_bass_kernel_spmd(nc, in_maps, core_ids=list(range(N_CORES)))
    _last_results = res
    out_s = np.stack([res.results[i]["out"] for i in range(N_CORES)])
    out = np.empty_like(out_s)
    out[:, perm] = out_s                      # undo the bn_inv row sort
    return out.reshape(N_CORES, Co, H, W).astype(F32)


if __name__ == "__main__":
    rng = np.random.default_rng(0)
    dummy = {
        "low_feats": rng.standard_normal((8, Cl, H, W), dtype=np.float32),
        "high_feats": rng.standard_normal((8, Ch, H, W), dtype=np.float32),
    }
    for k, shape in [("q_w", (Cm, Ch)), ("k_w", (Cm, Cl)), ("v_w", (Cm, Cl)),
                     ("o_w", (Co, Cm)), ("bn_w", (Co, Co + Ch))]:
        dummy[k] = rng.standard_normal(shape, dtype=np.float32) * 0.02
    for k in ["q_g", "q_v", "k_g", "k_v"]:
        dummy[k] = rng.uniform(0.5, 1.5, Cm).astype(np.float32)
    for k in ["q_b", "q_m", "k_b", "k_m", "v_b"]:
        dummy[k] = rng.standard_normal(Cm).astype(np.float32) * 0.1
    for k in ["bn_g", "bn_v"]:
        dummy[k] = rng.uniform(0.5, 1.5, Co).astype(np.float32)
    for k in ["bn_b", "bn_m", "o_b"]:
        dummy[k] = rng.standard_normal(Co).astype(np.float32) * 0.1
    out = kernel(**dummy)
    print("out", out.shape, out.dtype)


# revision 61
# speedup vs baseline: 1.0524x; 1.0524x over previous
"""AFNB (asymmetric fusion non-local block) Trainium2 kernel.

Data-parallel over batch: B=8 images, one per NeuronCore, no collectives.

Per-core algorithm (one image, H*W = N = 4096 pixels):
  pass 1 (low path):  kv = [relu(k_conv + kb); v_conv](low) computed
                      pixel-major in fp8 DoubleRow (2x PE rate); the K bias is
                      added by the DVE (psum + broadcast row) instead of a PE
                      matmul; pyramid-pooled (indicator matmul, fp8 DR) ->
                      kv_pool [110, 512] -> k_pool (PE transpose, stored fp8
                      at k/4) and v_poolT [110, 256] bf16.  V bias is folded
                      into the final output bias on the host (softmax rows sum
                      to 1, so it is exact).
  pass 2 (per 512-pixel tile): q = relu(q_conv(high)) in fp8 DR at q/4;
                      sim = (k/4).(q/4) (fp8 DR) = k.q/16 BN-folded; softmax
                      over the 110 pooled slots; ctx = v.softmax (bf16);
                      out = [W2_bf16 | W2_fp8]@high + A@ctx + bias with
                      A = bn_inv*(bn_w1@o_w)/32 and W2 = bn_inv*bn_w2 folded
                      on the host.

W2 mixed precision: output rows are sorted by bn_inv (host permutes rows of
W2/A/bias and un-permutes the result after the gather -- free).  Row chunk oc
(128 sorted rows) computes its K-suffix of PAIRS[oc]*256 input channels in fp8
DoubleRow (half the PE time of bf16) and the remaining prefix in bf16.  Low
bn_inv rows tolerate more fp8 (quantization error scales with the row norm);
the PAIRS allocation keeps the per-chunk max |error| <= ~1.45e-2 * out_scale
against the 2e-2 budget.  high is shipped twice: bf16 chunks 0..13 for the
bf16 matmuls and fp8(high*32) all 16 chunks for the q conv + fp8 W2 part
(also kills the on-chip bf16->fp8 cast that used to clog the DVE queue ahead
of psum drains).

Scale management (e4m3; subnormal below 2^-6): kw,vw,kb scaled x32; qw x256
and hi8 x32 (both undone by the q activation scale 1/32768); k_pool stored as
k/4 and q as q/4 so sim = k.q/16; psum scale for the out matmuls is
WSC = 8192 = (W2 bf16 x8192) = (W2 fp8 x256)@(hi8 x32) = (A x256)@(ctx x32).
Output stored fp16, upcast + row-unpermuted on the host.
"""

import numpy as np
import ml_dtypes

import concourse.bass as bass
import concourse.mybir as mybir
import concourse.tile as tile
from concourse import bacc
from concourse.bass_utils import run_bass_kernel_spmd
from concourse.masks import make_identity

BF = ml_dtypes.bfloat16
F8 = ml_dtypes.float8_e4m3fn
F16 = np.float16
F32 = np.float32
EPS = 1e-5
P = 128
N_CORES = 8
Cl, Ch, Cm, Co = 1024, 2048, 256, 2048
H = W = 64
NPIX = H * W            # 4096
M = 110                 # pooled slots: 1 + 9 + 36 + 64
M2 = 112                # M padded to a multiple of 16 (DoubleRow LDWEIGHTS stride rule)
KO = Cl // P            # 8  low-channel chunks
QO = Ch // P            # 16 high-channel chunks
OC = Co // P            # 16 out-channel chunks
PIX_T = 512             # pixel tile
NT = NPIX // PIX_T      # 8 tiles
PC = PIX_T // P         # 4 pixel chunks (128) per tile
KSC = 32.0              # K/V channel scale for fp8 range
QWS = 256.0             # q-weight fp8 scale
HISC = 32.0             # hi8 fp8 scale
AS8 = 256.0             # A fp8 scale
W8S = 256.0             # W2 fp8 scale (W8S * HISC = WSC)
WSC = 8192.0            # common psum scale for the out matmul group

# fp8 K-pairs (256 ch each) per sorted output chunk, low bn_inv first.  Tuned
# so each chunk's max W2-quantization error stays under ~1.45e-2 of out scale.
PAIRS = (8, 8, 8, 8, 7, 6, 4, 4, 4, 3, 3, 3, 2, 2, 2, 1)
NBF = tuple(16 - 2 * p for p in PAIRS)      # leading bf16 K chunks per oc
WOFF = tuple(int(x) for x in np.cumsum((0,) + NBF))   # into w2bf flat dim
# group processing order within a tile: natural order starts with the pure
# fp8 groups (oc 0-2), whose weights (W28 head) land first at startup, and
# reaches the bf16-heavy tail as the W2BF/hi_bf streams arrive
OCORD = tuple(range(16))
POFF = {}               # into w28 flat dim, laid out in OCORD order
_off = 0
for _oc in OCORD:
    POFF[_oc] = _off
    _off += PAIRS[_oc]
NW2BF = WOFF[-1]        # 130 bf16 weight chunks total
NW28 = _off             # 63 fp8 weight pair-chunks total
QOB = max(NBF)          # 14 bf16 hi chunks shipped

_cached = None
_last_results = None


def _pool_matrix(in_size, out_size):
    Pm = np.zeros((out_size, in_size), np.float64)
    for i in range(out_size):
        s = (i * in_size) // out_size
        e = -((-(i + 1) * in_size) // out_size)
        Pm[i, s:e] = 1.0 / (e - s)
    return Pm


def _build_pool_indicator():
    rows, areas = [], []
    for s in (1, 3, 6, 8):
        Ph = _pool_matrix(H, s) != 0
        Pw = _pool_matrix(W, s) != 0
        for i in range(s):
            for j in range(s):
                ind = np.outer(Ph[i], Pw[j]).reshape(-1)
                rows.append(ind.astype(np.float32))
                areas.append(ind.sum())
    return np.stack(rows), 1.0 / np.asarray(areas, np.float64)


def _chunk_T(w, chunks):
    """[rows, cols] -> SBUF layout [128, chunks, rows] with [p,o,m]=w[m,o*128+p]."""
    rows, cols = w.shape
    assert cols == chunks * P
    return np.ascontiguousarray(w.T.reshape(chunks, P, rows).transpose(1, 0, 2))


def _prep_weights(inp):
    f64 = lambda k: np.asarray(inp[k], np.float64)
    inv_q = f64("q_g") / np.sqrt(f64("q_v") + EPS)
    qw = (inv_q[:, None] * f64("q_w")) / 16.0        # fold BN + 1/sqrt(256)
    qb = (f64("q_b") - f64("q_m") * inv_q) / 16.0
    inv_k = f64("k_g") / np.sqrt(f64("k_v") + EPS)
    kw = inv_k[:, None] * f64("k_w")
    kb = f64("k_b") - f64("k_m") * inv_k
    bn_w1 = f64("bn_w")[:, :Ch]
    bn_w2 = f64("bn_w")[:, Ch:]
    inv_bn = f64("bn_g") / np.sqrt(f64("bn_v") + EPS)
    perm = np.argsort(inv_bn, kind="stable")         # rows sorted by bn_inv
    A = (inv_bn[:, None] * (bn_w1 @ f64("o_w")))[perm]       # [2048, 256]
    W2 = (inv_bn[:, None] * bn_w2)[perm]                     # [2048, 2048]
    # v_b folded here: ctx = softmax@(v+v_b) = softmax@v + v_b exactly
    bias_out = (inv_bn * (bn_w1 @ (f64("o_w") @ f64("v_b") + f64("o_b")))
                + f64("bn_b") - f64("bn_m") * inv_bn)[perm]

    kv_w = np.concatenate([kw, f64("v_w")], 0) * KSC  # [512, 1024] x32
    ind, area_recip = _build_pool_indicator()         # [110, 4096], [110]
    ind = np.concatenate([ind, np.zeros((M2 - M, NPIX), np.float32)], 0)

    # bf16 W2 prefix chunks, ragged per oc: [p, flat, c] = W2[oc*128+c, o*128+p]
    w2bf = np.zeros((P, NW2BF, P), np.float64)
    # fp8 W2 suffix pairs: [p, flatpair, s, m] = W2[oc*128+m, (2jj+s)*128+p]
    w28 = np.zeros((P, NW28, 2, P), np.float64)
    for oc in range(OC):
        rows = W2[oc * P:(oc + 1) * P]               # [128, 2048]
        for o in range(NBF[oc]):
            w2bf[:, WOFF[oc] + o, :] = rows[:, o * P:(o + 1) * P].T
        for j in range(PAIRS[oc]):
            jj = NBF[oc] // 2 + j
            for s in range(2):
                w28[:, POFF[oc] + j, s, :] = rows[:, (2 * jj + s) * P:(2 * jj + s + 1) * P].T

    return {
        "kvw8": _chunk_T(kv_w, KO).astype(F8),                        # [128, 8, 512]
        "kb": np.ascontiguousarray(np.stack([(kb * KSC), np.zeros_like(kb)])[None]).astype(F8),  # [1, 2, 256]
        "qw8": _chunk_T(qw * QWS, QO).astype(F8),                     # [128, 16, 256]
        "qb": np.ascontiguousarray((qb / 4.0).reshape(2, P).T).astype(F32),  # [128, 2]
        "AT": _chunk_T(A * AS8, 2).astype(F8),                        # [128, 2, 2048]
        "W2BF": np.ascontiguousarray(w2bf * WSC).astype(BF),          # [128, 130, 128]
        "W28": np.ascontiguousarray(w28 * W8S).astype(F8),            # [128, 63, 2, 128]
        "bout": np.ascontiguousarray(bias_out.reshape(OC, P).T).astype(F32),  # [128, 16]
        "ind8": _chunk_T(ind, NPIX // P).astype(F8),                  # [128, 32, 112]
        "area": np.ascontiguousarray(area_recip[:, None]).astype(F32),  # [110, 1]
    }, perm


def build_bass():
    bf = mybir.dt.bfloat16
    f8 = mybir.dt.float8e4
    f16 = mybir.dt.float16
    f32 = mybir.dt.float32
    DR = mybir.MatmulPerfMode.DoubleRow
    ACT = mybir.ActivationFunctionType
    ALU = mybir.AluOpType
    nc = bacc.Bacc()
    low_e = nc.declare_dram_parameter("low", [P, NT, KO, PIX_T], f8, isOutput=False)
    high_e = nc.declare_dram_parameter("high", [P, NT, QOB, PIX_T], bf, isOutput=False)
    hi8_e = nc.declare_dram_parameter("high8", [P, NT, 2, PIX_T], f8, isOutput=False)
    kvw_e = nc.declare_dram_parameter("kvw8", [P, KO, 512], f8, isOutput=False)
    kb_e = nc.declare_dram_parameter("kb", [1, 2, Cm], f8, isOutput=False)
    qw_e = nc.declare_dram_parameter("qw8", [P, QO, Cm], f8, isOutput=False)
    qb_e = nc.declare_dram_parameter("qb", [P, 2], f32, isOutput=False)
    at_e = nc.declare_dram_parameter("AT", [P, 2, Co], f8, isOutput=False)
    w2bf_e = nc.declare_dram_parameter("W2BF", [P, NW2BF, P], bf, isOutput=False)
    w28_e = nc.declare_dram_parameter("W28", [P, NW28, 2, P], f8, isOutput=False)
    bo_e = nc.declare_dram_parameter("bout", [P, OC], f32, isOutput=False)
    ind_e = nc.declare_dram_parameter("ind8", [P, NPIX // P, M2], f8, isOutput=False)
    ar_e = nc.declare_dram_parameter("area", [M, 1], f32, isOutput=False)
    out_e = nc.declare_dram_parameter("out", [Co, NPIX], f16, isOutput=True)

    out_r = out_e[:].rearrange("(o p) n -> o p n", p=P)    # [16, 128, 4096]

    with tile.TileContext(nc) as tc:
        with (
            tc.tile_pool(name="consts", bufs=1) as consts,
            tc.tile_pool(name="lobf", bufs=6) as lobf_p,
            tc.tile_pool(name="kvt", bufs=2) as kvt_p,
            tc.tile_pool(name="hibf", bufs=3) as hibf_p,
            tc.tile_pool(name="hi8b", bufs=3) as hi8_p,
            tc.tile_pool(name="qsb", bufs=2) as q_p,
            tc.tile_pool(name="esb", bufs=1) as e_p,
            tc.tile_pool(name="ensb", bufs=1) as en_p,
            tc.tile_pool(name="rsb", bufs=1) as r_p,
            tc.tile_pool(name="ctxsb", bufs=2) as ctx_p,
            tc.tile_pool(name="osb", bufs=4) as o_p,
            tc.tile_pool(name="psbig", bufs=2, space="PSUM") as psbig_p,
            tc.tile_pool(name="pso", bufs=5, space="PSUM") as pso_p,
            tc.tile_pool(name="psmall", bufs=1, space="PSUM") as psmall_p,
        ):
            # PE warmup: junk matmuls so the HAM clock gate opens (1.2->2.4GHz
            # after ~3.4us of activity) during the DMA-wait window at start
            junk = consts.tile([P, PIX_T], bf)
            nc.vector.memset(junk, 0.015)
            pswarm = psbig_p.tile([P, PIX_T], f32, tag="big")
            for _ in range(10):
                nc.tensor.matmul(pswarm, junk[:, 0:P], junk, start=True,
                                 stop=True, skip_group_check=True)

            # pass-1 streams issued up front, STRIPED across the three DMA
            # trigger queues (sync, scalar, gpsimd; each ~110 B/ns) in
            # need-order: kvw/lo0 pairs first, ind pair-chunks paced to the
            # in-order pool matmuls, later lo tiles behind.  Pre-hoisted
            # triggers are safe even on scalar: they execute before any
            # compute lands in its strict FIFO.
            kb_sb = consts.tile([1, 2, Cm], f8)
            nc.sync.dma_start(kb_sb, kb_e[:])
            kvw_sb = consts.tile([P, KO, 512], f8)
            lo_tiles = [lobf_p.tile([P, KO, PIX_T], f8, name="lo8")
                        for i in range(6)]
            ind_sb = consts.tile([P, NPIX // P, M2], f8)
            lo0 = lo_tiles[0]
            lo0_e = low_e[:][:, 0]
            kvw_d = kvw_e[:]
            nc.sync.dma_start(kvw_sb[:, 0:2, :], kvw_d[:, 0:2, :])
            nc.scalar.dma_start(kvw_sb[:, 2:4, :], kvw_d[:, 2:4, :])
            nc.gpsimd.dma_start(kvw_sb[:, 4:6, :], kvw_d[:, 4:6, :])
            nc.sync.dma_start(lo0[:, 0:2, :], lo0_e[:, 0:2, :])
            nc.scalar.dma_start(lo0[:, 2:4, :], lo0_e[:, 2:4, :])
            nc.gpsimd.dma_start(lo0[:, 4:6, :], lo0_e[:, 4:6, :])
            nc.sync.dma_start(kvw_sb[:, 6:8, :], kvw_d[:, 6:8, :])
            nc.scalar.dma_start(lo0[:, 6:8, :], lo0_e[:, 6:8, :])
            nc.gpsimd.dma_start(ind_sb[:, 0:4, :], ind_e[:][:, 0:4, :])
            nc.sync.dma_start(lo_tiles[1], low_e[:][:, 1])
            nc.scalar.dma_start(lo_tiles[2], low_e[:][:, 2])
            nc.gpsimd.dma_start(ind_sb[:, 4:12, :], ind_e[:][:, 4:12, :])
            nc.scalar.dma_start(lo_tiles[3], low_e[:][:, 3])
            nc.gpsimd.dma_start(ind_sb[:, 12:20, :], ind_e[:][:, 12:20, :])
            nc.sync.dma_start(lo_tiles[4], low_e[:][:, 4])
            nc.sync.dma_start(lo_tiles[5], low_e[:][:, 5])
            nc.gpsimd.dma_start(ind_sb[:, 20:32, :], ind_e[:][:, 20:32, :])
            ar_sb = consts.tile([M, 1], f32)
            nc.gpsimd.dma_start(ar_sb, ar_e[:])
            # tile-0 bf16 hi staged pre-loop: the scalar queue is clear of
            # compute here, and pass-1 emits no further scalar triggers
            hi_tiles, hi8_tiles = {}, {}

            def stage_hibf(t):
                hi_bf = hibf_p.tile([P, QOB, PIX_T], bf, name="hi_bf")
                nc.scalar.dma_start(hi_bf, high_e[:][:, t])
                hi_tiles[t] = hi_bf

            # tile-0 bf16 hi rides the scalar queue behind the lo stream;
            # needed only from tile-0's 4th group (oc3), well after it lands
            stage_hibf(0)

            ones8 = consts.tile([1, 2, P], f8)    # K-bias DR seed lhsT
            nc.vector.memset(ones8[:, 0, :], 1.0)
            nc.vector.memset(ones8[:, 1, :], 0.0)
            ones1m = consts.tile([1, M], bf)      # psr broadcast lhsT
            nc.vector.memset(ones1m, 1.0)
            ones_m = consts.tile([M, 1], bf)      # denominator lhsT
            nc.vector.memset(ones_m, 1.0)
            ident = consts.tile([P, P], f32)
            make_identity(nc, ident)

            kvpool_f32 = consts.tile([M, 512], f32)
            v_poolT = consts.tile([M, Cm], bf)
            k_pool8 = consts.tile([P, 2, M2], f8)
            nc.vector.memset(k_pool8, 0.0)

            # ---------------- pass 1: low -> pooled K/V (fp8 DR) ----------------
            pool_acc = pso_p.tile([M2, 512], f32, name="pool_acc", tag="o")
            kvt8 = None
            kvt_gate = None
            for dt_ in range(NT):
                lo8 = lo_tiles[dt_]
                # tiles 6/7 reuse ring slots 0/1 once those readers exist; the
                # triggers land on queues that are idle by then
                if dt_ == 1:
                    lo_tiles.append(lobf_p.tile([P, KO, PIX_T], f8, name="lo8"))
                    nc.sync.dma_start(lo_tiles[6], low_e[:][:, 6])
                elif dt_ == 2:
                    lo_tiles.append(lobf_p.tile([P, KO, PIX_T], f8, name="lo8"))
                    nc.gpsimd.dma_start(lo_tiles[7], low_e[:][:, 7])
                for tt in range(PC):
                    t = dt_ * PC + tt
                    ps = psbig_p.tile([P, 512], f32, tag="big")
                    for o2 in range(KO // 2):
                        nc.tensor.matmul(
                            ps, lo8[:, 2 * o2:2 * o2 + 2, tt * P:(tt + 1) * P],
                            kvw_sb[:, 2 * o2:2 * o2 + 2, :],
                            start=(o2 == 0), stop=(o2 == KO // 2 - 1),
                            perf_mode=DR, skip_group_check=True)
                        if o2 == 0:   # K-bias into the zeroed psum (cols 0:256)
                            nc.tensor.matmul(ps[:, 0:Cm], ones8[:, 0:2, :],
                                             kb_sb[:, 0:2, :],
                                             start=False, stop=False, perf_mode=DR,
                                             skip_group_check=True)
                    half = t % 2
                    if half == 0:
                        kvt8 = kvt_p.tile([P, 2, 512], f8)
                    nc.scalar.activation(kvt8[:, half, 0:Cm], ps[:, 0:Cm], ACT.Relu)
                    nc.vector.tensor_copy(kvt8[:, half, Cm:512], ps[:, Cm:512])
                    if half == 1:
                        nc.tensor.matmul(pool_acc, ind_sb[:, t - 1:t + 1, :], kvt8,
                                         start=(t == 1), stop=(t == NPIX // P - 1),
                                         perf_mode=DR, skip_group_check=True)
                        if t == 11:
                            kvt_gate = kvt8

            # hi8: only the fp8-exclusive chunks 14/15 come over DMA; chunks
            # 0..13 are cast on-chip from hi_bf (x32, bf16->fp8) in pieces
            # interleaved between groups so they never clog the DVE FIFO
            CAST_SL = ((0, 4), (4, 8), (8, 11), (11, 14))

            def stage_hi8(t):
                hi8t = hi8_p.tile([P, QO, PIX_T], f8, name="hi8t")
                nc.sync.dma_start(hi8t[:, 14:16, :], hi8_e[:][:, t])
                hi8_tiles[t] = hi8t

            def cast_hi8(t, piece):
                a, b = CAST_SL[piece]
                nc.vector.tensor_scalar_mul(hi8_tiles[t][:, a:b, :],
                                            hi_tiles[t][:, a:b, :], HISC)

            stage_hi8(0)

            # pass-2 weights in need-order: tile 0 walks groups in OCORD, so
            # W28 heads (OCORD[:7], pairs 0:45) go before the tail; the bf16
            # W2 stream is trickled through tile 0's body below
            hibf1 = hibf_p.tile([P, QOB, PIX_T], bf, name="hi_bf")
            nc.sync.dma_start(hibf1[:, 0:7, :], high_e[:][:, 1, 0:7, :])
            nc.scalar.dma_start(hibf1[:, 7:QOB, :], high_e[:][:, 1, 7:QOB, :])
            hi_tiles[1] = hibf1
            at_sb = consts.tile([P, 2, Co], f8)
            nc.sync.dma_start(at_sb, at_e[:])
            w2bf_sb = consts.tile([P, NW2BF, P], bf)
            nc.sync.dma_start(w2bf_sb[:, 0:6], w2bf_e[:][:, 0:6])
            nc.sync.dma_start(w2bf_sb[:, 6:22], w2bf_e[:][:, 6:22])
            qw_sb = consts.tile([P, QO, Cm], f8)
            nc.gpsimd.dma_start(qw_sb, qw_e[:])
            qb_sb = consts.tile([P, 2], f32)
            nc.gpsimd.dma_start(qb_sb, qb_e[:])
            bo_sb = consts.tile([P, OC], f32)
            nc.gpsimd.dma_start(bo_sb, bo_e[:])
            w28_sb = consts.tile([P, NW28, 2, P], f8)
            nc.gpsimd.dma_start(w28_sb[:, 0:24], w28_e[:][:, 0:24])
            nc.gpsimd.dma_start(w28_sb[:, 24:45], w28_e[:][:, 24:45])
            nc.gpsimd.dma_start(w28_sb[:, 45:NW28], w28_e[:][:, 45:NW28])
            nc.gpsimd.dma_start(w2bf_sb[:, 42:64], w2bf_e[:][:, 42:64])

            def emit_q(t):
                """fp8 DR q conv for tile t -> q/4 in fp8 [P, 2, PIX_T]."""
                hi8t = hi8_tiles[t]
                q8 = q_p.tile([P, 2, PIX_T], f8)
                for j in range(2):
                    psq = psbig_p.tile([P, PIX_T], f32, tag="big")
                    for o2 in range(QO // 2):
                        nc.tensor.matmul(
                            psq, qw_sb[:, 2 * o2:2 * o2 + 2, j * P:(j + 1) * P],
                            hi8t[:, 2 * o2:2 * o2 + 2, :],
                            start=(o2 == 0), stop=(o2 == QO // 2 - 1),
                            perf_mode=DR)
                    nc.scalar.activation(q8[:, j, :], psq, ACT.Relu,
                                         bias=qb_sb[:, j:j + 1],
                                         scale=1.0 / (4.0 * QWS * HISC))
                return q8

            for pc_ in range(4):
                cast_hi8(0, pc_)
            q_next = emit_q(0)

            # epilogue: scale by 1/area, split V (bf16) / K (fp8 at k/4 via
            # transpose + scaled copy); PE transposes overlap tile-0 q conv
            nc.vector.tensor_scalar_mul(kvpool_f32, pool_acc[0:M, :], ar_sb)
            nc.scalar.activation(v_poolT, kvpool_f32[:, Cm:512], ACT.Copy)
            for j in range(2):
                pst = psbig_p.tile([P, M], f32, tag="big")
                nc.tensor.transpose(pst, kvpool_f32[:, j * P:(j + 1) * P],
                                    ident[:M, :M])
                nc.scalar.activation(k_pool8[:, j, 0:M], pst, ACT.Identity,
                                     scale=1.0 / (4.0 * KSC))

            # per-oc W2 op lists: bf16 prefix chunks + fp8 DR suffix pairs
            OPS = []
            for oc in range(OC):
                OPS.append([("bf", o) for o in range(NBF[oc])]
                           + [("dr", NBF[oc] // 2 + j) for j in range(PAIRS[oc])])

            # ---------------- pass 2: per pixel tile ----------------
            # ctx for tile t+1 is produced inside tile t (chain links spread
            # between groups, hiding their serial latency behind queued PE
            # work), so each group runs contiguously: its W2 matmuls, the
            # A@ctx close, the split drain and the output DMA.  Psum banks
            # recycle a full 4-group period after their drain.

            def chain_head(q8):
                psim = psmall_p.tile([M2, PIX_T], f32, tag="s", name="psim")
                nc.tensor.matmul(psim, k_pool8[:, 0:2, :], q8[:, 0:2, :],
                                 start=True, stop=True, perf_mode=DR,
                                 skip_group_check=True)
                e_sb = e_p.tile([M, PIX_T], bf, name="e_sb")
                nc.scalar.activation(e_sb, psim[0:M, :], ACT.Exp)
                return e_sb

            def chain_mid1(e_sb):
                psd = psmall_p.tile([1, PIX_T], f32, tag="s", name="psd")
                nc.tensor.matmul(psd, ones_m, e_sb, start=True, stop=True,
                                 skip_group_check=True)
                r_sb = r_p.tile([1, PIX_T], f32, name="r_sb")
                nc.vector.reciprocal_approx_fast(out=r_sb, in_=psd)
                r_bf = r_p.tile([1, PIX_T], bf, name="r_bf")
                nc.scalar.activation(r_bf, r_sb, ACT.Copy)
                return r_bf

            def chain_mid2(e_sb, r_bf):
                psr = psmall_p.tile([M, PIX_T], f32, tag="s", name="psr")
                nc.tensor.matmul(psr, ones1m, r_bf, start=True, stop=True,
                                 skip_group_check=True)
                en_sb = en_p.tile([M, PIX_T], bf, name="en_sb")
                nc.vector.tensor_mul(en_sb, e_sb, psr)
                return en_sb

            def chain_tail1(en_sb):
                psc0 = psbig_p.tile([P, PIX_T], f32, tag="big", name="psc0")
                nc.tensor.matmul(psc0, v_poolT[:, 0:P], en_sb,
                                 start=True, stop=True, skip_group_check=True)
                psc1 = psbig_p.tile([P, PIX_T], f32, tag="big", name="psc1")
                nc.tensor.matmul(psc1, v_poolT[:, P:2 * P], en_sb,
                                 start=True, stop=True, skip_group_check=True)
                return psc0, psc1

            def chain_tail2(psc0, psc1):
                ctx_sb = ctx_p.tile([P, 2, PIX_T], f8, name="ctx_sb")
                nc.vector.tensor_copy(ctx_sb[:, 0, :], psc0)
                nc.vector.tensor_copy(ctx_sb[:, 1, :], psc1)
                return ctx_sb

            # tile-0 chain in the prologue (overlaps the pass-1 tail / q conv)
            e_nx = chain_head(q_next)
            r_nx = chain_mid1(e_nx)
            en_nx = chain_mid2(e_nx, r_nx)
            pc0, pc1 = chain_tail1(en_nx)
            ctx_next = chain_tail2(pc0, pc1)

            for t in range(NT):
                sl = slice(t * PIX_T, (t + 1) * PIX_T)
                hi_bf = hi_tiles[t]
                hi8t = hi8_tiles[t]
                if t + 2 < NT:
                    stage_hibf(t + 2)
                q8 = q_next
                ctx_sb = ctx_next
                last = t + 1 >= NT

                def do_group(oc, gi, hi_bf=hi_bf, hi8t=hi8t, ctx_sb=ctx_sb,
                             sl=sl, last=last):
                    pso = pso_p.tile([P, PIX_T], f32, name="pso", tag="o")
                    first = True
                    for kind, idx in OPS[oc]:
                        if kind == "bf":
                            nc.tensor.matmul(pso, w2bf_sb[:, WOFF[oc] + idx, :],
                                             hi_bf[:, idx, :],
                                             start=first, stop=False,
                                             skip_group_check=True)
                        else:
                            j = idx - NBF[oc] // 2
                            nc.tensor.matmul(pso, w28_sb[:, POFF[oc] + j],
                                             hi8t[:, 2 * idx:2 * idx + 2, :],
                                             start=first, stop=False,
                                             perf_mode=DR, skip_group_check=True)
                        first = False
                    nc.tensor.matmul(pso, at_sb[:, 0:2, oc * P:(oc + 1) * P],
                                     ctx_sb[:, 0:2, :],
                                     start=False, stop=True, perf_mode=DR,
                                     skip_group_check=True)
                    o_sb = o_p.tile([P, PIX_T], f16)
                    # full-width drains on alternating engines: keeps each
                    # FIFO sparse so the softmax-chain ops are not delayed
                    if gi % 2 == 1:
                        nc.scalar.activation(o_sb, pso, ACT.Identity,
                                             bias=bo_sb[:, oc:oc + 1],
                                             scale=1.0 / WSC)
                        if last and gi % 4 == 3:
                            nc.scalar.dma_start(out_r[oc][:, sl], o_sb)
                        else:
                            nc.gpsimd.dma_start(out_r[oc][:, sl], o_sb)
                    else:
                        nc.vector.tensor_scalar(o_sb, pso, 1.0 / WSC,
                                                bo_sb[:, oc:oc + 1],
                                                ALU.mult, ALU.add)
                        nc.sync.dma_start(out_r[oc][:, sl], o_sb)

                for gi, oc in enumerate(OCORD):
                    do_group(oc, gi)
                    if t == 0:   # trickle the bf16 W2 stream behind tile 0
                        if gi == 0:
                            nc.sync.dma_start(w2bf_sb[:, 22:42],
                                              w2bf_e[:][:, 22:42])
                        elif gi == 1:
                            nc.gpsimd.dma_start(w2bf_sb[:, 64:88],
                                                w2bf_e[:][:, 64:88])
                        elif gi == 2:
                            nc.sync.dma_start(w2bf_sb[:, 88:NW2BF],
                                              w2bf_e[:][:, 88:NW2BF])
                    if not last:
                        if gi == 0:
                            stage_hi8(t + 1)
                        elif 1 <= gi <= 4:
                            cast_hi8(t + 1, gi - 1)
                        elif gi == 5:
                            q_next = emit_q(t + 1)
                        elif gi == 6:
                            e_nx = chain_head(q_next)
                        elif gi == 10:
                            r_nx = chain_mid1(e_nx)
                        elif gi == 12:
                            en_nx = chain_mid2(e_nx, r_nx)
                        elif gi == 13:
                            pc0, pc1 = chain_tail1(en_nx)
                        elif gi == 14:
                            ctx_next = chain_tail2(pc0, pc1)
    nc.finalize()
    return nc


def kernel(**inputs):
    global _cached, _last_results
    if _cached is None:
        _cached = build_bass()
    nc = _cached
    wts, perm = _prep_weights(inputs)
    # pack [C, H*W] -> [p, tile, o, pix] so each per-tile DMA is contiguous
    low = np.ascontiguousarray(
        np.asarray(inputs["low_feats"], F32).reshape(N_CORES, KO, P, NT, PIX_T)
        .transpose(0, 2, 3, 1, 4).astype(F8))
    high_f = (np.asarray(inputs["high_feats"], F32)
              .reshape(N_CORES, QO, P, NT, PIX_T).transpose(0, 2, 3, 1, 4))
    high = np.ascontiguousarray(high_f[:, :, :, 0:QOB, :].astype(BF))
    high8 = np.ascontiguousarray((high_f[:, :, :, 14:16, :] * HISC).astype(F8))
    in_maps = [dict(wts, low=low[i], high=high[i], high8=high8[i])
               for i in range(N_CORES)]
    res = run_bass_kernel_spmd(nc, in_maps, core_ids=list(range(N_CORES)))
    _last_results = res
    out_s = np.stack([res.results[i]["out"] for i in range(N_CORES)])
    out = np.empty_like(out_s)
    out[:, perm] = out_s                      # undo the bn_inv row sort
    return out.reshape(N_CORES, Co, H, W).astype(F32)


if __name__ == "__main__":
    rng = np.random.default_rng(0)
    dummy = {
        "low_feats": rng.standard_normal((8, Cl, H, W), dtype=np.float32),
        "high_feats": rng.standard_normal((8, Ch, H, W), dtype=np.float32),
    }
    for k, shape in [("q_w", (Cm, Ch)), ("k_w", (Cm, Cl)), ("v_w", (Cm, Cl)),
                     ("o_w", (Co, Cm)), ("bn_w", (Co, Co + Ch))]:
        dummy[k] = rng.standard_normal(shape, dtype=np.float32) * 0.02
    for k in ["q_g", "q_v", "k_g", "k_v"]:
        dummy[k] = rng.uniform(0.5, 1.5, Cm).astype(np.float32)
    for k in ["q_b", "q_m", "k_b", "k_m", "v_b"]:
        dummy[k] = rng.standard_normal(Cm).astype(np.float32) * 0.1
    for k in ["bn_g", "bn_v"]:
        dummy[k] = rng.uniform(0.5, 1.5, Co).astype(np.float32)
    for k in ["bn_b", "bn_m", "o_b"]:
        dummy[k] = rng.standard_normal(Co).astype(np.float32) * 0.1
    out = kernel(**dummy)
    print("out", out.shape, out.dtype)
